# revision 1
# baseline (speedup 1.0000x reference)
"""Trainium2 Bass kernel for nn_Attention_interaction (dense_transformer).

Math (per batch b, head h):
    q = l2norm(x);  S = (q @ q^T) / SCALE / attn_gamma;  P = softmax(S, -1)
    o = P @ y;  o2 = o @ W^T + bias;  out = w0*y + w1*o2
with w_i = exp(sum_gamma_i) / (exp(sum_gamma0) + exp(sum_gamma1)).

Sharding: batch dim B=8 across the 8 cores (1 batch x 8 heads per core).
Per core the 8 heads run in 4 pairs (head A's qT operands on SBUF
partitions 0-63, head B's on 64-127, addressed via matmul tile_position).

The kernel is ACT(exp)-bound: softmax needs 8.4M exps per core and the
Scalar engine runs 1 elem/lane/cycle at 1.2 GHz (~55us floor). Everything
is arranged so the exp stream never waits and the PE stays dense (and
therefore HAM-warm):
  - Softmax skips max-subtraction (q rows are unit vectors so logits are
    bounded by 1/(SCALE*gamma)) and exp needs no accumulator: the softmax
    denominators accumulate in the O matmul's 65th output row via a
    ones-column appended to y on the host.
  - Per head the S columns are laid out jc-major (col = jc*4096 + i*512)
    and streamed through [128,1024] PSUM chunks (2-slot round robin, heads
    interleaved). O matmuls are emitted into the chunk loop with a
    one-chunk delay (2 per chunk, evenly) so the static per-engine program
    order is always runnable. PSUM plan (8 banks): S stream 4, per-head
    jc0-bank (O-jc0 then proj blocks 0-3) and jc1-bank (O-jc1 then proj
    blocks 4-7) = 4.
  - q-prep: l2norm via fast-inverse-sqrt + Newton on DVE (no Sqrt/Rsqrt
    tables — only Exp is used, one activation-table load), scale+cast to
    bf16, q^T built by DMA-xbar transposes (bacc's event-semaphore pass
    legalizes the XPOSE single-wait-slot limit).
  - proj = OT^T @ wt_aug with K=65: wt_aug row 64 = w1*bias, so r*w1*bias
    folds into the matmul and the 1/r epilogue scale leaves exactly
    w1*bias; epilogue adds the host-provided w0*y.
  - Denominator rows are moved into [128,1]-per-block layout by a small
    DRAM-bounce scatter DMA (DMA engines are otherwise idle).
"""

import math
import os

import numpy as np
import ml_dtypes

import concourse.bass as bass
import concourse.bacc as bacc
import concourse.tile as tile
from concourse import mybir
from concourse.bass_utils import run_bass_kernel_spmd
from concourse._compat import get_trn_type

B, H, N, D = 8, 8, 1024, 64
SCALE = (512 // 8) ** (-0.5)  # 0.125
EPS = 1e-6
NCORES = 8
NB = N // 128  # 8 row blocks of 128
NW = N * NB  # 8192 flattened S columns per head
CHUNK = 1024  # exp granularity (PSUM columns per ACT instruction)
F32 = mybir.dt.float32
BF16 = mybir.dt.bfloat16
I32 = mybir.dt.int32
AX = mybir.AxisListType
OP = mybir.AluOpType
ACT = mybir.ActivationFunctionType
MAGIC = 0x5F3759DF

LAST_RESULTS = None  # BassKernelResults of the most recent run (for test.py)


def _emit(ctx, tc, sqrt_c2: float):
    """Emit the per-core program. sqrt_c2 = sqrt(1/(SCALE*attn_gamma)) is
    folded into the q row scales so S comes out of the PE pre-scaled."""
    nc = tc.nc
    x_bf = nc.dram_tensor("x_bf", [H, N, D], BF16, kind="ExternalInput")
    ya = nc.dram_tensor("ya", [H, N, D + 1], BF16, kind="ExternalInput")
    yb = nc.dram_tensor("yb", [H, N, D], F32, kind="ExternalInput")
    wt = nc.dram_tensor("wt", [D + 1, D], BF16, kind="ExternalInput")
    out = nc.dram_tensor("out", [H, N, D], F32, kind="ExternalOutput")
    # DRAM bounce buffer for the denominator-row transposes
    rscr = nc.dram_tensor("rscr", [2, 2, N], BF16)

    singles = ctx.enter_context(tc.tile_pool(name="singles", bufs=1))
    io = ctx.enter_context(tc.tile_pool(name="io", bufs=2))
    st = ctx.enter_context(tc.tile_pool(name="st", bufs=2))
    work = ctx.enter_context(tc.tile_pool(name="work", bufs=2))
    epool = ctx.enter_context(tc.tile_pool(name="epool", bufs=2))
    qpool = ctx.enter_context(tc.tile_pool(name="qpool", bufs=1))
    # PSUM: 8 banks = S stream 2x[128,1024] (4) + per-head jc0/jc1 banks (4)
    ps_s = ctx.enter_context(tc.tile_pool(name="ps_s", bufs=2, space="PSUM"))
    ps_o = ctx.enter_context(tc.tile_pool(name="ps_o", bufs=1, space="PSUM"))

    # proj weight (rows 0-63 = w1*W^T, row 64 = w1*bias)
    wt_sb = singles.tile([D + 1, D], BF16)
    nc.sync.dma_start(out=wt_sb, in_=wt[:, :])

    qT = [None] * (H // 2)

    def prep(p):
        """Loads + l2norm + q scale/cast + DMA-transpose into qT[p].
        Processed in two block groups so pair 0's first S chunk (which only
        needs qT blocks 0-3) is ready as early as possible."""
        hA, hB = 2 * p, 2 * p + 1
        xA = io.tile([128, NB, D], BF16, tag="xA")
        xB = io.tile([128, NB, D], BF16, tag="xB")
        nc.sync.dma_start(out=xA, in_=x_bf[hA].rearrange("(b p) d -> p b d", p=128))
        nc.sync.dma_start(out=xB, in_=x_bf[hB].rearrange("(b p) d -> p b d", p=128))

        q = qpool.tile([128, N], BF16, tag=f"qT{p}", name=f"qT{p}")
        hb = NB // 2
        for g in range(2):
            b0 = g * hb
            # row norms for blocks b0..b0+3 of both heads:
            # ss[:, 0:4] = head A, ss[:, 4:8] = head B
            ss = st.tile([128, 2 * hb], F32, tag="ss")
            sqA = st.tile([128, hb, D], F32, tag="sqA")
            sqB = st.tile([128, hb, D], F32, tag="sqB")
            nc.vector.tensor_mul(sqA, xA[:, b0 : b0 + hb, :], xA[:, b0 : b0 + hb, :])
            nc.vector.reduce_sum(ss[:, 0:hb], sqA, axis=AX.X)
            nc.vector.tensor_mul(sqB, xB[:, b0 : b0 + hb, :], xB[:, b0 : b0 + hb, :])
            nc.vector.reduce_sum(ss[:, hb : 2 * hb], sqB, axis=AX.X)

            # rs = sqrt_c2 / sqrt(ss + eps): fast inverse sqrt + 3 Newton
            half = st.tile([128, 2 * hb], F32, tag="half")
            nc.vector.tensor_scalar(
                out=half, in0=ss, scalar1=0.5, scalar2=0.5 * EPS,
                op0=OP.mult, op1=OP.add,
            )
            yv = st.tile([128, 2 * hb], F32, tag="yv")
            yi = yv.bitcast(I32)
            nc.vector.tensor_scalar(
                out=yi, in0=ss.bitcast(I32), scalar1=1, scalar2=None,
                op0=OP.logical_shift_right,
            )
            nc.vector.tensor_scalar(
                out=yi, in0=yi, scalar1=MAGIC, scalar2=-1,
                op0=OP.subtract, op1=OP.mult,
            )
            t1 = st.tile([128, 2 * hb], F32, tag="t1")
            for it in range(3):
                last = it == 2
                nc.vector.tensor_mul(t1, yv, yv)
                nc.vector.tensor_mul(t1, t1, half)
                nc.vector.tensor_scalar(
                    out=t1, in0=t1, scalar1=1.5,
                    scalar2=(-sqrt_c2 if last else -1.0),
                    op0=OP.subtract, op1=OP.mult,
                )
                nc.vector.tensor_mul(yv, yv, t1)

            # q blocks (bf16), interleaved [A-dims | B-dims] per 128-col
            # group, then DMA-xbar transpose into qT
            qAB = work.tile([128, hb, 128], BF16, tag="qAB")
            for b in range(hb):
                nc.vector.tensor_scalar_mul(
                    out=qAB[:, b, 0:D], in0=xA[:, b0 + b, :],
                    scalar1=yv[:, b : b + 1],
                )
                nc.vector.tensor_scalar_mul(
                    out=qAB[:, b, D:128], in0=xB[:, b0 + b, :],
                    scalar1=yv[:, hb + b : hb + b + 1],
                )
            for b in range(hb):
                nc.sync.dma_start(
                    out=q[:, (b0 + b) * 128 : (b0 + b + 1) * 128],
                    in_=qAB[:, b],
                    transpose=True,
                )
        qT[p] = q

    prep(0)
    prep(1)

    for p in range(H // 2):
        hA, hB = 2 * p, 2 * p + 1
        q = qT[p]

        yA = io.tile([128, NB, D + 1], BF16, tag="yA")
        yB = io.tile([128, NB, D + 1], BF16, tag="yB")
        ybA = io.tile([128, NB, D], F32, tag="ybA")
        ybB = io.tile([128, NB, D], F32, tag="ybB")
        nc.sync.dma_start(out=yA, in_=ya[hA].rearrange("(b p) d -> p b d", p=128))
        nc.sync.dma_start(out=yB, in_=ya[hB].rearrange("(b p) d -> p b d", p=128))
        nc.sync.dma_start(out=ybA, in_=yb[hA].rearrange("(b p) d -> p b d", p=128))
        nc.sync.dma_start(out=ybB, in_=yb[hB].rearrange("(b p) d -> p b d", p=128))

        EA = epool.tile([128, NW], BF16, tag="EA")
        EB = epool.tile([128, NW], BF16, tag="EB")
        OTA = work.tile([D + 1, N], BF16, tag="OTA")
        OTB = work.tile([D + 1, N], BF16, tag="OTB")
        heads = (
            (0, EA, yA, OTA),
            (64, EB, yB, OTB),
        )
        okptr = [0, 0]  # per head: next O matmul (jc-major index jc*8+i)
        otile = [None, None]

        def emit_o(hidx, limit):
            """Emit O matmuls whose E input (cols < limit) is ready. The
            65th output row accumulates the softmax denominators."""
            base, E, ytile, OT = heads[hidx]
            hc = "AB"[hidx]
            while okptr[hidx] < 16:
                k = okptr[hidx]
                jc, i = k // NB, k % NB
                if jc * 4096 + (i + 1) * 512 > limit:
                    return
                if i == 0:
                    otile[hidx] = ps_o.tile(
                        [128, 512], F32, tag=f"o{jc}{hc}", name=f"ot{jc}{hc}"
                    )
                nc.tensor.matmul(
                    otile[hidx][0 : D + 1, :],
                    lhsT=ytile[:, i, :],
                    rhs=E[:, jc * 4096 + i * 512 : jc * 4096 + (i + 1) * 512],
                    start=(i == 0), stop=(i == NB - 1), tile_position=(0, 0),
                )
                if i == NB - 1:
                    nc.vector.tensor_copy(
                        OT[:, jc * 512 : (jc + 1) * 512],
                        otile[hidx][0 : D + 1, :],
                    )
                okptr[hidx] += 1

        def emit_proj(hidx, jc):
            """proj for output blocks jc*4..jc*4+3 (needs OT cols of that jc
            half); lands in the jc bank this head just freed."""
            base, E, ytile, OT = heads[hidx]
            hc = "AB"[hidx]
            pj = ps_o.tile([128, 512], F32, tag=f"o{jc}{hc}", name=f"pj{jc}{hc}")
            for b in range(jc * 4, jc * 4 + 4):
                nc.tensor.matmul(
                    pj[:, (b - jc * 4) * 128 : (b - jc * 4) * 128 + D],
                    lhsT=OT[:, b * 128 : (b + 1) * 128],
                    rhs=wt_sb,
                    start=True, stop=True, tile_position=(0, 0),
                )
            return pj

        pjs = [[None, None], [None, None]]  # [hidx][jc]
        # ---- S/exp chunk stream with O interleaved (one-chunk delay) ----
        for c in range(NW // CHUNK):
            jc, ip = c // 4, (c % 4) * 2
            for hidx, (base, E, ytile, OT) in enumerate(heads):
                ps = ps_s.tile([128, CHUNK], F32, tag="psS", name="psS")
                for i in (ip, ip + 1):
                    nc.tensor.matmul(
                        ps[:, (i - ip) * 512 : (i - ip + 1) * 512],
                        lhsT=q[base : base + 64, i * 128 : (i + 1) * 128],
                        rhs=q[base : base + 64, jc * 512 : (jc + 1) * 512],
                        start=True, stop=True, tile_position=(base, 0),
                    )
                nc.scalar.activation(
                    out=E[:, c * CHUNK : (c + 1) * CHUNK], in_=ps, func=ACT.Exp
                )
                emit_o(hidx, c * CHUNK)
                if c == 4:
                    # jc0 accumulation evacuated at c==4's emit_o; its bank
                    # is free — run the first proj half here.
                    pjs[hidx][0] = emit_proj(hidx, 0)

        # ---- pair tail: O flush, denominators, proj half 2, epilogue ----
        rT = st.tile([128, 2, NB], BF16, tag="rT")
        rinv = st.tile([128, 2 * NB], F32, tag="rinv")
        for hidx, (base, E, ytile, OT) in enumerate(heads):
            emit_o(hidx, NW)
            nc.sync.dma_start(out=rscr[p % 2, hidx], in_=OT[D : D + 1, :])
            nc.sync.dma_start(
                out=rT[:, hidx, :],
                in_=rscr[p % 2, hidx].rearrange("(b p) -> p b", p=128),
            )
            pjs[hidx][1] = emit_proj(hidx, 1)
        nc.vector.reciprocal(rinv, rT.rearrange("p a b -> p (a b)"))

        for hidx, o2t, ybt, fint, ho in (
            (0, "o2A", "ybA", "finA", hA),
            (1, "o2B", "ybB", "finB", hB),
        ):
            o2 = work.tile([128, NB, D], F32, tag=o2t, name=o2t)
            for b in range(NB):
                nc.vector.tensor_scalar_mul(
                    out=o2[:, b, :],
                    in0=pjs[hidx][b // 4][:, (b % 4) * 128 : (b % 4) * 128 + D],
                    scalar1=rinv[:, hidx * NB + b : hidx * NB + b + 1],
                )
            fin = work.tile([128, NB, D], F32, tag=fint, name=fint)
            nc.vector.tensor_add(fin, o2, ybA if hidx == 0 else ybB)
            nc.sync.dma_start(
                out=out[ho].rearrange("(b p) d -> p b d", p=128), in_=fin
            )

        if p + 2 < H // 2:
            prep(p + 2)


def build_program(sqrt_c2: float) -> bass.Bass:
    from contextlib import ExitStack

    nc = bacc.Bacc(get_trn_type() or "TRN2", target_bir_lowering=False)
    with tile.TileContext(nc) as tc:
        with ExitStack() as ctx:
            _emit(ctx, tc, sqrt_c2)
    # bacc passes legalize sync waits (≤1 wait per instruction on TRN2) and
    # insert the activation-table loads.
    nc.compile()
    return nc


def kernel(x, y, proj_w, proj_b, attn_gamma, sum_gamma0, sum_gamma1):
    global LAST_RESULTS
    x = np.asarray(x, dtype=np.float32)
    y = np.asarray(y, dtype=np.float32)
    proj_w = np.asarray(proj_w, dtype=np.float32)
    proj_b = np.asarray(proj_b, dtype=np.float32)
    g0 = math.exp(float(np.asarray(sum_gamma0)))
    g1 = math.exp(float(np.asarray(sum_gamma1)))
    w0 = g0 / (g0 + g1)
    w1 = g1 / (g0 + g1)
    c2 = 1.0 / (SCALE * float(np.asarray(attn_gamma)))

    nc = build_program(math.sqrt(c2))

    x_bf = x.astype(ml_dtypes.bfloat16)
    # y with a ones column appended: the O matmul's 65th output row then
    # accumulates the softmax denominators.
    ya = np.concatenate(
        [y, np.ones(y.shape[:-1] + (1,), np.float32)], axis=-1
    ).astype(ml_dtypes.bfloat16)
    yb = (w0 * y).astype(np.float32)
    # wt rows 0-63 = w1*W^T; row 64 = w1*bias (multiplies the r row, so the
    # 1/r epilogue scale leaves exactly w1*bias).
    wt = np.concatenate([proj_w.T * w1, w1 * proj_b[None, :]], axis=0).astype(
        ml_dtypes.bfloat16
    )

    in_maps = [
        {"x_bf": x_bf[c], "ya": ya[c], "yb": yb[c], "wt": wt}
        for c in range(NCORES)
    ]
    res = run_bass_kernel_spmd(nc, in_maps, list(range(NCORES)))
    LAST_RESULTS = res
    return np.stack([res.results[c]["out"] for c in range(NCORES)], axis=0)

